# revision 12
# baseline (speedup 1.0000x reference)
"""Trainium2 Bass kernel for nn_BrainInspiredAttention.

Sharding: 8 cores = (B=2) x (4 sequence blocks of W=1024). Each core
computes q for its own block, recomputes k/v for (prev block + own block)
strip locally (zero communication), runs blocked sliding-window attention
for its block, and the output projection for its 1024 rows.

All matmuls bf16 (fp32 matmul is 4x slower on TRN2 PE), fp32 PSUM accum.

Layouts (per core):
  xT   [C=2048, T2=2048]  x^T of the strip (prev block zeros for blk 0)
  kT   spilled to DRAM [H, 128(d), T2]: rope'd, un-normalized (rms factor
       folded into exp's per-partition scale), reloaded per head
  qTn  [128(d), H, TQ=1024] transposed, rope'd + rms-normalized queries
  v    spilled to DRAM [T2, C] (gated ve added), reloaded per head
  S^T  [kk, i] score tiles -> exp -> P^T in SBUF (multiplicative masks)
  O^T  [128(d), H, TQ] accumulated via lhsT=v_h tiles; denominator via
       ones-vector matmul (per-core data zeroes prev-block for blk 0)
  out  = (O^T/den).T @ Wproj  [TQ, C] fp32
"""

import sys

sys.path.insert(0, "/opt/trn_rl_repo")

from contextlib import ExitStack

import numpy as np
import ml_dtypes

import concourse.bass as bass
import concourse.mybir as mybir
import concourse.tile as tile
from concourse import bacc
from concourse.bass_utils import run_bass_kernel_spmd

BF16 = mybir.dt.bfloat16
F32 = mybir.dt.float32
AF = mybir.ActivationFunctionType
OP = mybir.AluOpType

B, T, C, H, D = 2, 4096, 2048, 16, 128
W = 1024          # window / block size
NB = T // W       # 4 blocks
N_CORES = 8
T2 = 2 * W        # strip length (prev + own block)
TQ = W            # queries per core
CT = C // 128     # 16 contraction tiles
EPS = 1e-6

# score kk-tiles for i-chunk ic (512 queries): kt in [4*ic, 4*ic+11]
N_SLOT = 12


def _masked_kts(ic):
    """kt values whose S^T tile needs a multiplicative mask op (uniform
    across cores; block-0 handling is via data: ones_in + zeroed x/ve)."""
    if ic == 0:
        return [0, 1, 2, 3, 8, 9, 10, 11]
    return [4, 5, 6, 7, 12, 13, 14, 15]


def _mask_idx(ic, kt):
    s = kt - 4 * ic
    return s if s < 4 else s - 4


def build_kernel():
    nc = bacc.Bacc("TRN2", target_bir_lowering=False, debug=False,
                   num_devices=N_CORES)

    xT = nc.dram_tensor("xT", [C, T2], BF16, kind="ExternalInput")
    veb = nc.dram_tensor("veb", [T2, C], BF16, kind="ExternalInput")
    # ccat = [cos; cos], ssig = [+sin; -sin] stacked along d (128 partitions)
    cosT = nc.dram_tensor("cosT", [128, T2], BF16, kind="ExternalInput")
    sinT = nc.dram_tensor("sinT", [128, T2], BF16, kind="ExternalInput")
    Wq = nc.dram_tensor("Wq", [C, C], BF16, kind="ExternalInput")
    Wk = nc.dram_tensor("Wk", [C, C], BF16, kind="ExternalInput")
    Wv = nc.dram_tensor("Wv", [C, C], BF16, kind="ExternalInput")
    Wp = nc.dram_tensor("Wp", [C, C], BF16, kind="ExternalInput")
    Wg = nc.dram_tensor("Wg", [32, H], BF16, kind="ExternalInput")
    ones_in = nc.dram_tensor("ones_in", [128, CT], BF16, kind="ExternalInput")
    masks = nc.dram_tensor("masks", [2, 8, 128, 512], BF16,
                           kind="ExternalInput")
    out = nc.dram_tensor("out", [TQ, C], F32, kind="ExternalOutput")

    vspill = nc.dram_tensor("vspill", [T2, C], BF16)
    kspill = nc.dram_tensor("kspill", [H, 128, T2], BF16)

    with tile.TileContext(nc) as tc, ExitStack() as top:
        persist = top.enter_context(tc.tile_pool(name="persist", bufs=1))

        qt_sb = persist.tile([128, H, TQ], BF16)           # 4 MB
        ot_sb = persist.tile([128, H, TQ], BF16)           # 4 MB
        ones_sb = persist.tile([128, CT], BF16)
        nc.sync.dma_start(out=ones_sb, in_=ones_in[:, :])
        rkbuf = persist.tile([128, H, CT], F32)
        eps_sb = persist.tile([128, 1], F32)
        nc.vector.memset(eps_sb, EPS)
        epsd_sb = persist.tile([128, 1], F32)
        nc.vector.memset(epsd_sb, float(D) * EPS)

        with ExitStack() as xphase:
            xpool = xphase.enter_context(tc.tile_pool(name="xt", bufs=1))
            xt_sb = xpool.tile([128, CT, T2], BF16)        # 8 MB
            nc.sync.dma_start(out=xt_sb,
                              in_=xT.rearrange("(ct p) t -> p ct t", p=128))
            cos_sb = xpool.tile([128, T2], BF16)
            sin_sb = xpool.tile([128, T2], BF16)
            nc.sync.dma_start(out=cos_sb, in_=cosT[:, :])
            nc.sync.dma_start(out=sin_sb, in_=sinT[:, :])

            # ---------- phase A: gate + v (spilled to DRAM) ----------
            with ExitStack() as ph:
                wpool = ph.enter_context(tc.tile_pool(name="wA", bufs=2))
                work = ph.enter_context(tc.tile_pool(name="workA", bufs=3))
                gpool = ph.enter_context(tc.tile_pool(name="gate", bufs=1))
                psA = ph.enter_context(tc.tile_pool(name="psA", bufs=2, space="PSUM"))
                psG = ph.enter_context(tc.tile_pool(name="psG", bufs=2, space="PSUM"))

                wg_sb = gpool.tile([32, H], BF16)
                nc.sync.dma_start(out=wg_sb, in_=Wg[:, :])
                gate_sb = gpool.tile([128, T2 // 128, H], BF16)
                # gate: sigmoid(x @ Wg); the factor 2 is folded into ve on host
                for tt in range(T2 // 128):
                    g_ps = psG.tile([128, H], F32)
                    nc.tensor.matmul(g_ps,
                                     xt_sb[0:32, 0, tt * 128:(tt + 1) * 128],
                                     wg_sb, start=True, stop=True)
                    nc.scalar.activation(out=gate_sb[:, tt, :], in_=g_ps,
                                         func=AF.Sigmoid)

                wvr = Wv.rearrange("(ct p) m -> p ct m", p=128)
                for cc in range(4):          # c_out chunks of 512
                    wv_sb = wpool.tile([128, CT, 512], BF16, tag="wA")
                    nc.sync.dma_start(out=wv_sb,
                                      in_=wvr[:, :, cc * 512:(cc + 1) * 512])
                    for tt in range(T2 // 128):
                        v_ps = psA.tile([128, 512], F32)
                        for ct in range(CT):
                            nc.tensor.matmul(
                                v_ps, xt_sb[:, ct, tt * 128:(tt + 1) * 128],
                                wv_sb[:, ct, :],
                                start=(ct == 0), stop=(ct == CT - 1))
                        v_sb = work.tile([128, 512], BF16, tag="vsb")
                        nc.scalar.activation(out=v_sb, in_=v_ps, func=AF.Copy)
                        ve_sb = work.tile([128, 512], BF16, tag="vesb")
                        nc.sync.dma_start(
                            out=ve_sb,
                            in_=veb[tt * 128:(tt + 1) * 128,
                                    cc * 512:(cc + 1) * 512])
                        # gv = gate (broadcast over d) * ve
                        g2d = gate_sb[:, tt, cc * 4:(cc + 1) * 4]
                        g_b = bass.AP(g2d.tensor, g2d.offset,
                                      [g2d.ap[0], g2d.ap[1], [0, 128]])
                        gv = work.tile([128, 4, 128], BF16, tag="gvsb")
                        nc.vector.tensor_mul(
                            gv, ve_sb.rearrange("p (h d) -> p h d", d=128), g_b)
                        nc.vector.tensor_add(v_sb, v_sb,
                                             gv.rearrange("p h d -> p (h d)"))
                        nc.sync.dma_start(
                            out=vspill[tt * 128:(tt + 1) * 128,
                                       cc * 512:(cc + 1) * 512],
                            in_=v_sb)

            # ---------- phase B/C: kT (spill) and qTn ----------
            def proj_rope(wten, n_chunks, t_off, is_q):
                with ExitStack() as ph:
                    wpool = ph.enter_context(tc.tile_pool(name="wB", bufs=2))
                    work = ph.enter_context(tc.tile_pool(name="workB", bufs=3))
                    psB = ph.enter_context(tc.tile_pool(name="psB", bufs=2, space="PSUM"))
                    psR = ph.enter_context(tc.tile_pool(name="psR", bufs=2, space="PSUM"))
                    wr = wten.rearrange("(ct p) m -> p ct m", p=128)
                    for h in range(H):
                        w_sb = wpool.tile([128, CT, 128], BF16, tag="wB")
                        nc.sync.dma_start(out=w_sb,
                                          in_=wr[:, :, h * 128:(h + 1) * 128])
                        for ch in range(n_chunks):
                            sl = slice(ch * 512, (ch + 1) * 512)
                            sl_abs = slice(t_off + ch * 512,
                                           t_off + (ch + 1) * 512)
                            p_ps = psB.tile([128, 512], F32)
                            for ct in range(CT):
                                nc.tensor.matmul(p_ps, w_sb[:, ct, :],
                                                 xt_sb[:, ct, sl_abs],
                                                 start=(ct == 0),
                                                 stop=(ct == CT - 1))
                            raw = work.tile([128, 512], BF16, tag="raw")
                            nc.scalar.activation(out=raw, in_=p_ps, func=AF.Copy)
                            # rope: rop = raw*[c;c] + swap(raw)*[s;-s]
                            swp = work.tile([128, 512], BF16, tag="swp")
                            nc.sync.dma_start(out=swp[0:64, :], in_=raw[64:128, :])
                            nc.sync.dma_start(out=swp[64:128, :], in_=raw[0:64, :])
                            t1 = work.tile([128, 512], BF16, tag="t1")
                            t2 = work.tile([128, 512], BF16, tag="t2")
                            rop = work.tile([128, 512], BF16, tag="rop")
                            nc.vector.tensor_mul(t1, raw, cos_sb[:, sl_abs])
                            nc.vector.tensor_mul(t2, swp, sin_sb[:, sl_abs])
                            nc.vector.tensor_add(rop, t1, t2)
                            sq = work.tile([128, 512], BF16, tag="sq")
                            nc.vector.tensor_mul(sq, rop, rop)
                            if not is_q:
                                nc.sync.dma_start(out=kspill[h, :, sl], in_=rop)
                                # rms scale per kk: rk = 1/sqrt(sumsq/D + eps)
                                ssq = psR.tile([128, 4], F32)
                                for j in range(4):
                                    nc.tensor.matmul(
                                        ssq[:, j:j + 1],
                                        sq[:, j * 128:(j + 1) * 128],
                                        ones_sb[:, CT - 1:CT],
                                        start=True, stop=True)
                                rst = work.tile([128, 4], F32, tag="rst")
                                nc.scalar.activation(out=rst, in_=ssq,
                                                     func=AF.Sqrt,
                                                     scale=1.0 / D, bias=eps_sb)
                                nc.vector.reciprocal(
                                    out=rkbuf[:, h, ch * 4:(ch + 1) * 4],
                                    in_=rst)
                            else:
                                # bq = exp(-.5 ln(sumsq + D*eps)) = rsq/sqrt(D)
                                zq = psR.tile([1, 512], F32)
                                nc.tensor.matmul(zq, ones_sb[:, CT - 1:CT], sq,
                                                 start=True, stop=True)
                                lnq = work.tile([1, 512], F32, tag="lnq")
                                nc.scalar.activation(out=lnq, in_=zq,
                                                     func=AF.Ln,
                                                     bias=epsd_sb[0:1, :])
                                lnb = work.tile([128, 512], F32, tag="lnb")
                                nc.gpsimd.partition_broadcast(lnb, lnq,
                                                              channels=128)
                                bq = work.tile([128, 512], BF16, tag="bq")
                                nc.scalar.activation(out=bq, in_=lnb,
                                                     func=AF.Exp, scale=-0.5)
                                nc.vector.tensor_mul(qt_sb[:, h, sl], rop, bq)

            proj_rope(Wk, 4, 0, is_q=False)
            proj_rope(Wq, 2, W, is_q=True)

        # ---------- phase D: attention ----------
        with ExitStack() as ph:
            vpool = ph.enter_context(tc.tile_pool(name="vh", bufs=2))
            kpool = ph.enter_context(tc.tile_pool(name="kh", bufs=2))
            mpool = ph.enter_context(tc.tile_pool(name="masksb", bufs=1))
            work = ph.enter_context(tc.tile_pool(name="workD", bufs=4))
            psS = ph.enter_context(tc.tile_pool(name="psS", bufs=3, space="PSUM"))
            psO = ph.enter_context(tc.tile_pool(name="psO", bufs=2, space="PSUM"))
            psDen = ph.enter_context(tc.tile_pool(name="psDen", bufs=2, space="PSUM"))

            m_sb = mpool.tile([128, 16, 512], BF16)
            nc.sync.dma_start(out=m_sb,
                              in_=masks.rearrange("a s p f -> p (a s) f"))

            vsr = vspill.rearrange("(n p) c -> p n c", p=128)
            for h in range(H):
                v_h = vpool.tile([128, T2 // 128, 128], BF16, tag="vh")
                nc.sync.dma_start(out=v_h,
                                  in_=vsr[:, :, h * 128:(h + 1) * 128])
                k_h = kpool.tile([128, T2], BF16, tag="kh")
                nc.sync.dma_start(out=k_h, in_=kspill[h, :, :])
                for ic in range(2):
                    kts = list(range(4 * ic, 4 * ic + N_SLOT))
                    msl = _masked_kts(ic)
                    o_ps = psO.tile([128, 512], F32)
                    den_ps = psDen.tile([1, 512], F32)
                    for idx, kt in enumerate(kts):
                        s_ps = psS.tile([128, 512], F32)
                        nc.tensor.matmul(
                            s_ps, k_h[:, kt * 128:(kt + 1) * 128],
                            qt_sb[:, h, ic * 512:(ic + 1) * 512],
                            start=True, stop=True)
                        pt = work.tile([128, 512], BF16, tag="pt")
                        nc.scalar.activation(out=pt, in_=s_ps, func=AF.Exp,
                                             scale=rkbuf[:, h, kt:kt + 1])
                        if kt in msl:
                            nc.vector.tensor_mul(
                                pt, pt,
                                m_sb[:, ic * 8 + _mask_idx(ic, kt), :])
                        first, last = idx == 0, idx == len(kts) - 1
                        nc.tensor.matmul(o_ps, v_h[:, kt, :], pt,
                                         start=first, stop=last)
                        nc.tensor.matmul(den_ps, ones_sb[:, kt:kt + 1], pt,
                                         start=first, stop=last)
                    # normalize: O / den via exp(-ln den) broadcast
                    lnd = work.tile([1, 512], F32, tag="lnd")
                    nc.scalar.activation(out=lnd, in_=den_ps, func=AF.Ln)
                    lnb = work.tile([128, 512], F32, tag="lnbD")
                    nc.gpsimd.partition_broadcast(lnb, lnd, channels=128)
                    rec = work.tile([128, 512], F32, tag="rec")
                    nc.scalar.activation(out=rec, in_=lnb, func=AF.Exp,
                                         scale=-1.0)
                    nc.vector.tensor_mul(ot_sb[:, h, ic * 512:(ic + 1) * 512],
                                         o_ps, rec)

        # ---------- phase E: output projection ----------
        with ExitStack() as ph:
            wpool = ph.enter_context(tc.tile_pool(name="wE", bufs=2))
            work = ph.enter_context(tc.tile_pool(name="workE", bufs=3))
            psE = ph.enter_context(tc.tile_pool(name="psE", bufs=2, space="PSUM"))
            wr = Wp.rearrange("(ct p) m -> p ct m", p=128)
            for cc in range(4):
                wp_sb = wpool.tile([128, CT, 512], BF16, tag="wE")
                nc.sync.dma_start(out=wp_sb, in_=wr[:, :, cc * 512:(cc + 1) * 512])
                for tt in range(TQ // 128):
                    f_ps = psE.tile([128, 512], F32)
                    for ct in range(CT):
                        nc.tensor.matmul(
                            f_ps, ot_sb[:, ct, tt * 128:(tt + 1) * 128],
                            wp_sb[:, ct, :], start=(ct == 0), stop=(ct == CT - 1))
                    f_sb = work.tile([128, 512], F32, tag="fsb")
                    nc.scalar.activation(out=f_sb, in_=f_ps, func=AF.Copy)
                    nc.sync.dma_start(
                        out=out[tt * 128:(tt + 1) * 128, cc * 512:(cc + 1) * 512],
                        in_=f_sb)

    nc.compile()
    return nc


_NC = None


def _get_nc():
    global _NC
    if _NC is None:
        _NC = build_kernel()
    return _NC


def _make_masks():
    """Uniform multiplicative masks (window + causal edges only)."""
    m = np.zeros((2, 8, 128, 512), np.float32)
    for ic in range(2):
        for kt in _masked_kts(ic):
            kk = (kt * 128 + np.arange(128))[:, None]      # strip key pos
            ii = (ic * 512 + np.arange(512))[None, :]      # query pos in block
            valid = (kk >= ii) & (kk <= ii + W)
            m[ic, _mask_idx(ic, kt)] = valid.astype(np.float32)
    return m.astype(ml_dtypes.bfloat16)


def kernel(x, ve, cos, sin, Wq, Wk, Wv, Wproj, Wg, window_size):
    assert int(window_size) == W
    nc = _get_nc()
    bf = ml_dtypes.bfloat16

    wq = np.asarray(Wq, np.float32).astype(bf)
    wk = np.asarray(Wk, np.float32).astype(bf)
    wv = np.asarray(Wv, np.float32).astype(bf)
    wp = np.asarray(Wproj, np.float32).astype(bf)
    wg = np.asarray(Wg, np.float32).astype(bf)
    masks = _make_masks()
    x = np.asarray(x, np.float32)
    ve = np.asarray(ve, np.float32)
    cos = np.asarray(cos, np.float32)
    sin = np.asarray(sin, np.float32)

    # cos/sin tables padded so strip positions < 0 get identity rotation
    cos_pad = np.concatenate([np.ones((W, D // 2), np.float32), cos], 0)
    sin_pad = np.concatenate([np.zeros((W, D // 2), np.float32), sin], 0)
    ccat = np.concatenate([cos_pad, cos_pad], 1)        # [W+T, 128]
    ssig = np.concatenate([sin_pad, -sin_pad], 1)

    in_maps = []
    for core in range(N_CORES):
        b, blk = core // NB, core % NB
        lo = blk * W - W
        xs = np.zeros((T2, C), np.float32)
        vs = np.zeros((T2, C), np.float32)
        if blk == 0:
            xs[W:] = x[b, 0:W]
            vs[W:] = 2.0 * ve[b, 0:W]
        else:
            xs[:] = x[b, lo:lo + T2]
            vs[:] = 2.0 * ve[b, lo:lo + T2]
        ones = np.ones((128, CT), np.float32)
        if blk == 0:
            ones[:, 0:8] = 0.0
        cs = ccat[lo + W:lo + W + T2].T       # [128, T2]
        sn = ssig[lo + W:lo + W + T2].T
        in_maps.append({
            "xT": np.ascontiguousarray(xs.T).astype(bf),
            "veb": vs.astype(bf),
            "cosT": np.ascontiguousarray(cs).astype(bf),
            "sinT": np.ascontiguousarray(sn).astype(bf),
            "Wq": wq, "Wk": wk, "Wv": wv, "Wp": wp, "Wg": wg,
            "ones_in": ones.astype(bf),
            "masks": masks,
        })

    res = run_bass_kernel_spmd(nc, in_maps, list(range(N_CORES)))
    outs = res.results
    full = np.zeros((B, T, C), np.float32)
    for core in range(N_CORES):
        b, blk = core // NB, core % NB
        full[b, blk * W:(blk + 1) * W] = outs[core]["out"]
    return full
